# revision 6
# baseline (speedup 1.0000x reference)
"""Trainium2 Bass kernel for nn_NewellGRUModel (B=512, S=1024, F=16, H=64).

Model (matches the jax reference):
  x = inputs[:, :, :15]; delta = inputs[:, :, 15]
  h = GRU(x)            # Keras reset_after=True, gate order (z, r, h)
  state = h_final + T[0] * mean_t(delta)
  out = BN(relu(state @ w1 + b1)) @ w2 + b2        # [B, 1]

Mapping: data-parallel across 8 NeuronCores (64 batch rows per core).
On-chip layout is transposed: gate/hidden dims on SBUF partitions,
batch on the free axis, so per-step biases fold into the matmuls and
weights contract along partitions.

Per group of 8 timesteps, two PSUM banks [128, 512] are pre-filled by
K=16 matmuls with the input-side projections (bias rows folded in via a
ones-feature):
  zr bank   rows 0:128 = [-(xz+bz) | xr+br]   (z negated -> sigmoid gives 1-z)
  rhxh bank rows 0:64  = b_rh  (recurrent h-gate part, prefill = bias)
            rows 64:128 = xh + b_ih
Each step accumulates the h-dependent matmuls into its 64-column slice,
then:   (zbar|r) = sigmoid(zr_slice)                  [one ACT op]
        p = r * rh_slice ; s = p + xh_slice
        sp = sigmoid(2s)  (= (tanh(s)+1)/2)           [same ACT table set]
        h' = h - zbar*(1+h) + 2*zbar*sp
All activations are Sigmoid/Relu => a single activation table set for
the whole kernel.
"""

import numpy as np

B, S, F, H = 512, 1024, 16, 64
NCORES = 8
BC = B // NCORES          # 64 batch per core
BN_EPS = 1e-3
TCH = 256                 # timesteps per x DMA chunk
GRP = 8                   # timesteps per psum prefill group
NGRP = S // GRP           # 128
PREF_AHEAD = 3            # prefill this many groups ahead

_CACHE = {}


def _split_sync_waits(nc, mybir, max_waits=1):
    """This container's walrus build rejects instructions carrying more
    than one sync-wait command.  Move excess waits onto same-engine NOPs
    inserted immediately before the instruction (engines execute their
    stream in order, so the semantics are identical)."""
    for fn in nc.m.functions:
        for blk in fn.blocks:
            out = []
            changed = False
            for inst in blk.instructions:
                si = inst.sync_info
                if si is not None and len(si.on_wait) > max_waits:
                    waits = list(si.on_wait)
                    for w in waits[max_waits:]:
                        nop = mybir.InstNoOp(
                            name=nc.get_next_instruction_name(), ins=[], outs=[]
                        )
                        nop.engine = inst.engine
                        nop.sync_info = mybir.SyncInfo(on_wait=[w], on_update=[])
                        out.append(nop)
                    inst.sync_info = mybir.SyncInfo(
                        on_wait=waits[:max_waits], on_update=list(si.on_update)
                    )
                    changed = True
                out.append(inst)
            if changed:
                blk.instructions = out


def _build():
    """Build the Bass module (shared by all 8 cores)."""
    import concourse.bass as bass
    import concourse.mybir as mybir
    from concourse.tile import TileContext
    from concourse.alu_op_type import AluOpType as ALU

    fp32 = mybir.dt.float32
    AF = mybir.ActivationFunctionType
    AX = mybir.AxisListType

    nc = bass.Bass("TRN2", num_devices=NCORES)

    xT = nc.dram_tensor("xT", [F, S * BC], fp32, kind="ExternalInput")
    dl = nc.dram_tensor("dl", [BC, S], fp32, kind="ExternalInput")
    wpre_zr_d = nc.dram_tensor("wpre_zr", [F, 2 * H], fp32, kind="ExternalInput")
    wpre_rhxh_d = nc.dram_tensor("wpre_rhxh", [F, 2 * H], fp32, kind="ExternalInput")
    wr_zr_d = nc.dram_tensor("wr_zr", [H, 2 * H], fp32, kind="ExternalInput")
    wr_h_d = nc.dram_tensor("wr_h", [H, H], fp32, kind="ExternalInput")
    w1aug_d = nc.dram_tensor("w1aug", [H + 2, 64], fp32, kind="ExternalInput")
    w2aug_d = nc.dram_tensor("w2aug", [65, 1], fp32, kind="ExternalInput")
    tsc_d = nc.dram_tensor("tsc", [1, 1], fp32, kind="ExternalInput")
    ident_d = nc.dram_tensor("ident", [H, H], fp32, kind="ExternalInput")
    y_d = nc.dram_tensor("y", [1, BC], fp32, kind="ExternalOutput")

    with TileContext(nc) as tc:
        with (
            tc.tile_pool(name="const", bufs=1) as cpool,
            tc.tile_pool(name="xchunk", bufs=2) as xpool,
            tc.tile_pool(name="work", bufs=3) as wpool,
            tc.tile_pool(name="hpool", bufs=2) as hpool,
            tc.tile_pool(name="pz", bufs=4, space="PSUM") as pz_pool,
            tc.tile_pool(name="ph", bufs=4, space="PSUM") as ph_pool,
        ):
            def cload(dram, shape, tag):
                t = cpool.tile(shape, fp32, tag=tag)
                nc.sync.dma_start(out=t[:], in_=dram[:])
                return t

            wpre_zr = cload(wpre_zr_d, [F, 2 * H], "wpre_zr")
            wpre_rhxh = cload(wpre_rhxh_d, [F, 2 * H], "wpre_rhxh")
            wr_zr = cload(wr_zr_d, [H, 2 * H], "wr_zr")
            wr_h = cload(wr_h_d, [H, H], "wr_h")
            w1aug = cload(w1aug_d, [H + 2, 64], "w1aug")
            w2aug = cload(w2aug_d, [65, 1], "w2aug")
            tsc = cload(tsc_d, [1, 1], "tsc")
            ident = cload(ident_d, [H, H], "ident")
            dl_sb = cload(dl, [BC, S], "dl")

            chunks = {}

            def get_chunk(c):
                if c not in chunks:
                    t = xpool.tile([F, TCH * BC], fp32, tag="xc")
                    nc.sync.dma_start(
                        out=t[:], in_=xT[:, c * TCH * BC:(c + 1) * TCH * BC]
                    )
                    chunks[c] = t
                return chunks[c]

            zr_banks = [None] * NGRP
            ph_banks = [None] * NGRP

            def prefill(g):
                zb = pz_pool.tile([128, GRP * BC], fp32, tag="zr")
                hb = ph_pool.tile([128, GRP * BC], fp32, tag="rhxh")
                zr_banks[g] = zb
                ph_banks[g] = hb
                c = (g * GRP) // TCH
                col0 = ((g * GRP) % TCH) * BC
                rhs = get_chunk(c)[:, col0:col0 + GRP * BC]
                nc.tensor.matmul(zb[:], wpre_zr[:], rhs,
                                 start=True, stop=False, skip_group_check=True)
                nc.tensor.matmul(hb[:], wpre_rhxh[:], rhs,
                                 start=True, stop=False, skip_group_check=True)

            h_cur = hpool.tile([H, BC], fp32, tag="h")
            nc.vector.memset(h_cur[:], 0.0)

            for g in range(PREF_AHEAD):
                prefill(g)

            for t in range(S):
                g, sl = divmod(t, GRP)
                if sl == 0 and g + PREF_AHEAD < NGRP:
                    pass  # prefill below, after this step's matmuls
                zb = zr_banks[g]
                hb = ph_banks[g]
                zr_sl = zb[:, sl * BC:(sl + 1) * BC]
                rh_sl = hb[0:H, sl * BC:(sl + 1) * BC]
                xh_sl = hb[H:2 * H, sl * BC:(sl + 1) * BC]

                nc.tensor.matmul(zr_sl, wr_zr[:], h_cur[:],
                                 start=False, stop=True, skip_group_check=True)
                nc.tensor.matmul(rh_sl, wr_h[:], h_cur[:],
                                 start=False, stop=True, skip_group_check=True)
                if sl == 0 and g + PREF_AHEAD < NGRP:
                    prefill(g + PREF_AHEAD)

                zr_g = wpool.tile([2 * H, BC], fp32, tag="zrg")
                nc.scalar.activation(zr_g[:], zr_sl, AF.Sigmoid)
                zbar = zr_g[0:H, :]
                rr = zr_g[H:2 * H, :]

                p = wpool.tile([H, BC], fp32, tag="p")
                nc.vector.tensor_tensor(out=p[:], in0=rr, in1=rh_sl, op=ALU.mult)
                s = wpool.tile([H, BC], fp32, tag="s")
                nc.vector.tensor_tensor(out=s[:], in0=p[:], in1=xh_sl, op=ALU.add)

                # off-chain (uses only zbar, h): w2t = h - zbar*(1+h)
                a1 = wpool.tile([H, BC], fp32, tag="a1")
                nc.vector.tensor_scalar_add(a1[:], h_cur[:], 1.0)
                a2 = wpool.tile([H, BC], fp32, tag="a2")
                nc.vector.tensor_tensor(out=a2[:], in0=zbar, in1=a1[:], op=ALU.mult)
                w2t = wpool.tile([H, BC], fp32, tag="w2t")
                nc.vector.tensor_tensor(out=w2t[:], in0=h_cur[:], in1=a2[:],
                                        op=ALU.subtract)

                sp = wpool.tile([H, BC], fp32, tag="sp")
                nc.scalar.activation(sp[:], s[:], AF.Sigmoid, scale=2.0)

                m2 = wpool.tile([H, BC], fp32, tag="m2")
                nc.vector.tensor_tensor(out=m2[:], in0=zbar, in1=sp[:], op=ALU.mult)
                h_new = hpool.tile([H, BC], fp32, tag="h")
                nc.vector.scalar_tensor_tensor(
                    out=h_new[:], in0=m2[:], scalar=2.0, in1=w2t[:],
                    op0=ALU.mult, op1=ALU.add,
                )
                h_cur = h_new

            # ---- epilogue: delta effect + dense head ----
            dsum = wpool.tile([BC, 1], fp32, tag="dsum")
            nc.vector.tensor_reduce(dsum[:], dl_sb[:], axis=AX.X, op=ALU.add)
            pt = pz_pool.tile([128, GRP * BC], fp32, tag="zr")
            nc.tensor.transpose(pt[0:1, 0:BC], dsum[:], ident[:])

            rhs_aug = wpool.tile([H + 2, BC], fp32, tag="rhsaug")
            nc.vector.memset(rhs_aug[:], 1.0)  # row 65 stays all-ones
            nc.vector.tensor_copy(out=rhs_aug[0:H, :], in_=h_cur[:])
            nc.vector.tensor_scalar_mul(rhs_aug[H:H + 1, :], pt[0:1, 0:BC],
                                        tsc[0:1, 0:1])

            yps = ph_pool.tile([128, GRP * BC], fp32, tag="rhxh")
            nc.tensor.matmul(yps[0:64, 0:BC], w1aug[:], rhs_aug[:],
                             start=True, stop=True, skip_group_check=True)
            r1aug = wpool.tile([65, BC], fp32, tag="r1aug")
            nc.vector.memset(r1aug[:], 1.0)  # row 64 stays all-ones
            nc.scalar.activation(r1aug[0:64, :], yps[0:64, 0:BC], AF.Relu)

            ops_ = pz_pool.tile([128, GRP * BC], fp32, tag="zr")
            nc.tensor.matmul(ops_[0:1, 0:BC], w2aug[:], r1aug[:],
                             start=True, stop=True, skip_group_check=True)
            y_sb = wpool.tile([1, BC], fp32, tag="ysb")
            nc.vector.tensor_copy(out=y_sb[:], in_=ops_[0:1, 0:BC])
            nc.sync.dma_start(out=y_d[:], in_=y_sb[:])

    _split_sync_waits(nc, mybir)
    return nc


def _prep_inputs(inputs):
    """Host-side reshape/shard + weight folding. Returns in_maps for 8 cores."""
    x = np.asarray(inputs["inputs"], dtype=np.float32)        # [B, S, 16]
    K = np.asarray(inputs["gru_kernel"], dtype=np.float32)    # [15, 192]
    R = np.asarray(inputs["gru_rec_kernel"], dtype=np.float32)  # [64, 192]
    bias = np.asarray(inputs["gru_bias"], dtype=np.float32)   # [2, 192]
    w1 = np.asarray(inputs["w1"], dtype=np.float32)
    b1 = np.asarray(inputs["b1"], dtype=np.float32)
    gam = np.asarray(inputs["bn_gamma"], dtype=np.float32)
    bet = np.asarray(inputs["bn_beta"], dtype=np.float32)
    mu = np.asarray(inputs["bn_mean"], dtype=np.float32)
    var = np.asarray(inputs["bn_var"], dtype=np.float32)
    w2 = np.asarray(inputs["w2"], dtype=np.float32)
    b2 = np.asarray(inputs["b2"], dtype=np.float32)
    T = np.asarray(inputs["T"], dtype=np.float32)

    bz = bias[0, 0:64] + bias[1, 0:64]
    br = bias[0, 64:128] + bias[1, 64:128]
    b_ih = bias[0, 128:192]
    b_rh = bias[1, 128:192]

    wpre_zr = np.zeros((F, 2 * H), np.float32)
    wpre_zr[:15, 0:64] = -K[:, 0:64]
    wpre_zr[15, 0:64] = -bz
    wpre_zr[:15, 64:128] = K[:, 64:128]
    wpre_zr[15, 64:128] = br

    wpre_rhxh = np.zeros((F, 2 * H), np.float32)
    wpre_rhxh[15, 0:64] = b_rh
    wpre_rhxh[:15, 64:128] = K[:, 128:192]
    wpre_rhxh[15, 64:128] = b_ih

    wr_zr = np.concatenate([-R[:, 0:64], R[:, 64:128]], axis=1)  # [64, 128]
    wr_h = np.ascontiguousarray(R[:, 128:192])                    # [64, 64]

    g2 = gam / np.sqrt(var + BN_EPS)
    w2p = g2 * w2[:, 0]
    b2p = float((bet - mu * g2) @ w2[:, 0] + b2[0])
    w1aug = np.concatenate([w1, w1.sum(0, keepdims=True), b1[None, :]], axis=0)
    w2aug = np.concatenate([w2p, [b2p]]).astype(np.float32)[:, None]  # [65, 1]
    tsc = np.array([[T[0] / S]], np.float32)
    ident = np.eye(H, dtype=np.float32)

    shared = dict(wpre_zr=wpre_zr, wpre_rhxh=wpre_rhxh, wr_zr=wr_zr, wr_h=wr_h,
                  w1aug=w1aug, w2aug=w2aug, tsc=tsc, ident=ident)

    in_maps = []
    for c in range(NCORES):
        xc = x[c * BC:(c + 1) * BC]                 # [64, S, 16]
        xT = np.empty((F, S, BC), np.float32)
        xT[:15] = xc[:, :, :15].transpose(2, 1, 0)  # [15, S, 64]
        xT[15] = 1.0
        dlc = np.ascontiguousarray(xc[:, :, 15])    # [64, S]
        m = dict(shared)
        m["xT"] = xT.reshape(F, S * BC)
        m["dl"] = dlc
        in_maps.append(m)
    return in_maps


def kernel(**inputs) -> np.ndarray:
    from concourse.bass_utils import run_bass_kernel_spmd

    if "nc" not in _CACHE:
        _CACHE["nc"] = _build()
    nc = _CACHE["nc"]
    in_maps = _prep_inputs(inputs)
    res = run_bass_kernel_spmd(nc, in_maps, core_ids=list(range(NCORES)))
    out = np.concatenate([res.results[c]["y"].reshape(BC) for c in range(NCORES)])
    return out.astype(np.float32)[:, None]          # [512, 1]
